# revision 22
# baseline (speedup 1.0000x reference)
"""Trainium2 Bass kernel for DenseConv2d.

Conv2d: input (32,128,56,56) f32, weight (256,128,3,3) f32, bias (256,) f32,
stride 1, pad 1, dilation 1 -> output (32,256,56,56) f32.

Strategy: data-parallel over batch across 8 NeuronCores (4 images per core).
Per core the conv is 9 accumulated matmuls (one per kernel tap) into PSUM:
out[co, pix] += W[kh,kw][ci,co].T @ x_pad[ci, shifted pix window].
Operands are bfloat16 (cast host-side): bf16 lowers to LDWEIGHTS+MATMUL
pairs with fast-weight-load that pipeline through the PE reorder window,
so matmuls run at the 448-cycle fill limit. PSUM accumulation stays fp32.

Loop nest is tap-outer over groups of row-blocks (up to 4 PSUM banks
accumulate concurrently). While one group's banks drain (DVE bias-add ->
SBUF -> HBM store on alternating DMA queues), the next group's matmuls
fill the other banks. The first (img0, cot0) pass uses chunk-aligned
groups so the PE never stalls on input DMA at startup, and the final pass
ends in a single-block group whose two halves drain via DVE and ACT in
parallel, keeping the post-stream tail short.

Input is chunked (row-blocks + halo per DMA) in exactly the order the
startup groups consume it; a cold warmup chain bridges the PE from the
framework preamble to the first chunk's arrival (~3us cold-queue DMA
latency) so the HAM clock-gate reaches 2.4 GHz before real work starts.
Layout prep (padding, channel-major transpose, bf16 cast) is host-side.
"""

import sys

if "/opt/trn_rl_repo" not in sys.path:
    sys.path.insert(0, "/opt/trn_rl_repo")

import numpy as np

N_CORES = 8
N, CI, H, W = 32, 128, 56, 56
CO, KH, KW = 256, 3, 3
NP_CORE = N // N_CORES          # images per core
HP, WP = H + 2, W + 2           # padded spatial dims
COT = CO // 128                 # out-channel tiles of 128
RB = 8                          # output rows per matmul block
NBLK = H // RB                  # row blocks per image
NCH = 4                         # chunks per image (first one is short)
N_WARMUP = 6                    # full-width PE warmup matmuls (~373ns cold)
N_WARMUP_SMALL = 4              # half-width tail warmups (~187ns cold)

_CACHE = {}


def _build_program():
    import concourse.mybir as mybir
    from concourse import bacc
    from concourse.tile import TileContext

    nc = bacc.Bacc(None, target_bir_lowering=False)

    x_d = nc.dram_tensor("x", [CI, NP_CORE, HP, WP], mybir.dt.bfloat16,
                         kind="ExternalInput")
    w_d = nc.dram_tensor("w", [CI, COT, KH * KW, 128], mybir.dt.bfloat16,
                         kind="ExternalInput")
    b_d = nc.dram_tensor("b2", [128, COT], mybir.dt.float32,
                         kind="ExternalInput")
    y_d = nc.dram_tensor("y", [COT, 128, NP_CORE, H, W], mybir.dt.float32,
                         kind="ExternalOutput")

    f32 = mybir.dt.float32
    bf16 = mybir.dt.bfloat16

    with TileContext(nc) as tc:
        with (
            tc.tile_pool(name="xin", bufs=1) as xpool,
            tc.tile_pool(name="wpool", bufs=1) as wpool,
            tc.tile_pool(name="bpool", bufs=1) as bpool,
            tc.tile_pool(name="psum", bufs=8, space="PSUM") as ppool,
            tc.tile_pool(name="out", bufs=6) as opool,
        ):
            # PE warmup on scratch data, concurrent with the first input
            # DMAs: bridges PE-free (post-preamble) to data-ready and puts
            # busy time on the HAM clock-gate window. memset rides gpsimd,
            # whose preamble drains before the PE's, so the first warmup
            # issues the moment the PE is free.
            scratch = xpool.tile([CI, RB * W], bf16, tag="scratch")
            nc.gpsimd.memset(scratch, 0.0)
            wups = ppool.tile([128, RB * W], f32, tag="ps")
            for _ in range(N_WARMUP):
                nc.tensor.matmul(wups, scratch[:, 0:128], scratch,
                                 start=True, stop=True)
            # Tapered tail: bridges PE-busy up to the first input chunk's
            # arrival (~10.3us, cold-queue DMA latency) in finer steps, so
            # the HAM activity window never sees an idle gap and the real
            # stream starts at 2.4 GHz.
            for _ in range(N_WARMUP_SMALL):
                nc.tensor.matmul(wups[:, 0:224], scratch[:, 0:128],
                                 scratch[:, 0:224], start=True, stop=True)

            # Weights split by out-channel tile; cot0 lands as two DMAs
            # (taps 0-4 / 5-9) spread over both queues so the first matmul
            # group only waits for taps 0-4 plus the first input chunk.
            w0 = wpool.tile([CI, KH * KW, 128], bf16, tag="w0")
            w1 = wpool.tile([CI, KH * KW, 128], bf16, tag="w1")
            bt = bpool.tile([128, COT], f32)

            def wslice(pos, cot):
                return w0[:, pos, :] if cot == 0 else w1[:, pos, :]

            # Input chunks per image: (padded_row0, n_blocks). The first is
            # a single block so the very first matmul group's data arrives
            # fast; block b lives in chunk CHMAP[b] at local row CHLOC[b].
            CHUNKS = [(0, 1), (RB, 2), (3 * RB, 2), (5 * RB, 2)]
            CHMAP, CHLOC = {}, {}
            b = 0
            for ci_, (r0_, nb_) in enumerate(CHUNKS):
                for j in range(nb_):
                    CHMAP[b], CHLOC[b] = ci_, j * RB
                    b += 1
            xt = {}

            def x_chunk_dma(img, ch, eng):
                r0, nb = CHUNKS[ch]
                rows = min(nb * RB + 2, HP - r0)
                t = xpool.tile([CI, rows, WP], bf16, tag=f"x{img}_{ch}")
                eng.dma_start(out=t, in_=x_d[:, img, r0:r0 + rows, :])
                xt[img, ch] = t

            # Critical path: img0/cot0 runs chunk-aligned groups [0],[1,2],
            # [3,4],[5,6]. The sync queue carries that chain in exactly the
            # order it is consumed (chunk0, taps5-8, chunk1, chunk2, cot1
            # weights); scalar serves taps0-4 + bias + chunk3 in parallel.
            x_chunk_dma(0, 0, nc.sync)
            nc.scalar.dma_start(out=w0[:, 0:5, :], in_=w_d[:, 0, 0:5, :])
            nc.sync.dma_start(out=w0[:, 5:9, :], in_=w_d[:, 0, 5:9, :])
            nc.scalar.dma_start(out=bt, in_=b_d[:, :])
            # chunk1 split across both queues: rows 0-9 serve blk1, rows
            # 10-17 complete blk2, halving its arrival time.
            r0c1, nbc1 = CHUNKS[1]
            rows_c1 = nbc1 * RB + 2
            t_c1 = xpool.tile([CI, rows_c1, WP], bf16, tag="x0_1")
            nc.sync.dma_start(out=t_c1[:, 0:10, :],
                              in_=x_d[:, 0, r0c1:r0c1 + 10, :])
            nc.scalar.dma_start(out=t_c1[:, 10:rows_c1, :],
                               in_=x_d[:, 0, r0c1 + 10:r0c1 + rows_c1, :])
            xt[0, 1] = t_c1
            x_chunk_dma(0, 2, nc.sync)
            x_chunk_dma(0, 3, nc.scalar)
            nc.sync.dma_start(out=w1, in_=w_d[:, 1, :, :])
            for img in range(1, NP_CORE):
                for ch in range(len(CHUNKS)):
                    x_chunk_dma(img, ch,
                                nc.scalar if (img + ch) % 2 else nc.sync)

            # Tap-outer over groups of row-blocks: one weight load per tap
            # per group feeds len(grp) back-to-back matmuls. 4+3 banks per
            # (img, cot) pass; the final pass ends in a single-block group
            # so the drain after the last matmul is short.
            store_q = [nc.sync, nc.scalar]
            nstore = 0

            for img in range(NP_CORE):
                for cot in range(COT):
                    first_pass = (img == 0 and cot == 0)
                    last_pass = (img == NP_CORE - 1 and cot == COT - 1)
                    if first_pass:
                        # Chunk-aligned so each group only waits for the
                        # next DMA chunk, never stalling the PE at startup.
                        groups = [[0], [1, 2], [3, 4], [5, 6]]
                    elif last_pass:
                        groups = [[0, 1, 2, 3], [4, 5], [6]]
                    else:
                        groups = [[0, 1, 2, 3], [4, 5, 6]]
                    for grp in groups:
                        pss = [ppool.tile([128, RB, W], f32, tag="ps",
                                          name=f"ps{j}")
                               for j in range(len(grp))]
                        for pos in range(KH * KW):
                            kh, kw = divmod(pos, KW)
                            wsl = wslice(pos, cot)
                            for j, blk in enumerate(grp):
                                ch, r0 = CHMAP[blk], CHLOC[blk]
                                rhs = xt[img, ch][:, r0 + kh:r0 + kh + RB,
                                                  kw:kw + W]
                                nc.tensor.matmul(
                                    pss[j], wsl, rhs,
                                    start=(pos == 0),
                                    stop=(pos == KH * KW - 1),
                                )
                        last_grp = last_pass and grp[0] == 6
                        for j, blk in enumerate(grp):
                            if last_grp:
                                # Tail: ship the final block as two
                                # half-copies, bias-added on two different
                                # engines concurrently and stored on both
                                # queues (store DMAs have ~0.6us fixed
                                # cost, so split no finer).
                                h = RB // 2
                                for q in range(2):
                                    otq = opool.tile([128, h, W], f32,
                                                     tag=f"otq{q}",
                                                     name=f"otq{q}")
                                    half = pss[j][:, q * h:(q + 1) * h, :]
                                    if q == 0:
                                        nc.vector.tensor_scalar_add(
                                            otq, half, bt[:, cot:cot + 1])
                                    else:
                                        # ACT engine drains this half in
                                        # parallel with the DVE one.
                                        nc.scalar.add(otq, half,
                                                      bt[:, cot:cot + 1])
                                    r = blk * RB + q * h
                                    store_q[q % 2].dma_start(
                                        out=y_d[cot, :, img, r:r + h, :],
                                        in_=otq)
                            else:
                                ot = opool.tile([128, RB, W], f32)
                                nc.vector.tensor_scalar_add(
                                    ot, pss[j], bt[:, cot:cot + 1])
                                store_q[nstore % 2].dma_start(
                                    out=y_d[cot, :, img,
                                            blk * RB:blk * RB + RB, :],
                                    in_=ot)
                                nstore += 1

    nc.compile()
    return nc


def prep_in_maps(input, weight, bias):
    """Host-side layout prep -> one in_map per core."""
    import ml_dtypes

    bf = ml_dtypes.bfloat16
    xp = np.pad(input, ((0, 0), (0, 0), (1, 1), (1, 1))).astype(bf)
    # weight [co, ci, kh, kw] -> [ci, cot, (kh kw), cop]
    wr = np.ascontiguousarray(
        weight.transpose(1, 2, 3, 0).reshape(CI, KH * KW, COT, 128)
        .transpose(0, 2, 1, 3)).astype(bf)
    b2 = np.ascontiguousarray(bias.reshape(COT, 128).T)

    in_maps = []
    for c in range(N_CORES):
        xc = np.ascontiguousarray(
            xp[c * NP_CORE:(c + 1) * NP_CORE].transpose(1, 0, 2, 3))
        in_maps.append({"x": xc, "w": wr, "b2": b2})
    return in_maps


def kernel(input, weight, bias):
    input = np.asarray(input, dtype=np.float32)
    weight = np.asarray(weight, dtype=np.float32)
    bias = np.asarray(bias, dtype=np.float32)

    if "nc" not in _CACHE:
        _CACHE["nc"] = _build_program()
    nc = _CACHE["nc"]

    from concourse.bass_utils import run_bass_kernel_spmd

    in_maps = prep_in_maps(input, weight, bias)
    res = run_bass_kernel_spmd(nc, in_maps, core_ids=list(range(N_CORES)))

    out = np.empty((N, CO, H, W), dtype=np.float32)
    for c in range(N_CORES):
        y = res.results[c]["y"]  # [COT, 128, NP_CORE, H, W]
        out[c * NP_CORE:(c + 1) * NP_CORE] = (
            y.transpose(2, 0, 1, 3, 4).reshape(NP_CORE, CO, H, W))
    return out


# revision 23
# speedup vs baseline: 1.2039x; 1.2039x over previous
"""Trainium2 Bass kernel for DenseConv2d.

Conv2d: input (32,128,56,56) f32, weight (256,128,3,3) f32, bias (256,) f32,
stride 1, pad 1, dilation 1 -> output (32,256,56,56) f32.

Strategy: data-parallel over batch across 8 NeuronCores (4 images per core).
Per core the conv is 9 accumulated matmuls (one per kernel tap) into PSUM:
out[co, pix] += W[kh,kw][ci,co].T @ x_pad[ci, shifted pix window].
Operands are bfloat16 (cast host-side): bf16 lowers to LDWEIGHTS+MATMUL
pairs with fast-weight-load that pipeline through the PE reorder window,
so matmuls run at the 448-cycle fill limit. PSUM accumulation stays fp32.

Loop nest is tap-outer over groups of row-blocks (up to 4 PSUM banks
accumulate concurrently). While one group's banks drain (DVE bias-add ->
SBUF -> HBM store on alternating DMA queues), the next group's matmuls
fill the other banks. The first (img0, cot0) pass uses chunk-aligned
groups so the PE never stalls on input DMA at startup, and the final pass
ends in a single-block group whose two halves drain via DVE and ACT in
parallel, keeping the post-stream tail short.

Input is chunked (row-blocks + halo per DMA) in exactly the order the
startup groups consume it; a cold warmup chain bridges the PE from the
framework preamble to the first chunk's arrival (~3us cold-queue DMA
latency) so the HAM clock-gate reaches 2.4 GHz before real work starts.
Layout prep (padding, channel-major transpose, bf16 cast) is host-side.
"""

import sys

if "/opt/trn_rl_repo" not in sys.path:
    sys.path.insert(0, "/opt/trn_rl_repo")

import numpy as np

N_CORES = 8
N, CI, H, W = 32, 128, 56, 56
CO, KH, KW = 256, 3, 3
NP_CORE = N // N_CORES          # images per core
HP, WP = H + 2, W + 2           # padded spatial dims
COT = CO // 128                 # out-channel tiles of 128
RB = 8                          # output rows per matmul block
NBLK = H // RB                  # row blocks per image
NCH = 4                         # chunks per image (first one is short)
N_WARMUP = 6                    # full-width PE warmup matmuls (~373ns cold)
N_WARMUP_SMALL = 4              # half-width tail warmups (~187ns cold)

_CACHE = {}


def _build_program():
    import concourse.mybir as mybir
    from concourse import bacc
    from concourse.tile import TileContext

    nc = bacc.Bacc(None, target_bir_lowering=False)

    x_d = nc.dram_tensor("x", [CI, NP_CORE, HP, WP], mybir.dt.bfloat16,
                         kind="ExternalInput")
    w_d = nc.dram_tensor("w", [CI, COT, KH * KW, 128], mybir.dt.bfloat16,
                         kind="ExternalInput")
    b_d = nc.dram_tensor("b2", [128, COT], mybir.dt.float32,
                         kind="ExternalInput")
    y_d = nc.dram_tensor("y", [COT, 128, NP_CORE, H, W], mybir.dt.float32,
                         kind="ExternalOutput")

    f32 = mybir.dt.float32
    bf16 = mybir.dt.bfloat16

    with TileContext(nc) as tc:
        with (
            tc.tile_pool(name="xin", bufs=1) as xpool,
            tc.tile_pool(name="wpool", bufs=1) as wpool,
            tc.tile_pool(name="bpool", bufs=1) as bpool,
            tc.tile_pool(name="psum", bufs=8, space="PSUM") as ppool,
            tc.tile_pool(name="out", bufs=6) as opool,
        ):
            # PE warmup on scratch data, concurrent with the first input
            # DMAs: bridges PE-free (post-preamble) to data-ready and puts
            # busy time on the HAM clock-gate window. memset rides gpsimd,
            # whose preamble drains before the PE's, so the first warmup
            # issues the moment the PE is free.
            scratch = xpool.tile([CI, RB * W], bf16, tag="scratch")
            nc.gpsimd.memset(scratch, 0.0)
            wups = ppool.tile([128, RB * W], f32, tag="ps")
            for _ in range(N_WARMUP):
                nc.tensor.matmul(wups, scratch[:, 0:128], scratch,
                                 start=True, stop=True)
            # Tapered tail: bridges PE-busy up to the first input chunk's
            # arrival (~10.3us, cold-queue DMA latency) in finer steps, so
            # the HAM activity window never sees an idle gap and the real
            # stream starts at 2.4 GHz.
            for _ in range(N_WARMUP_SMALL):
                nc.tensor.matmul(wups[:, 0:224], scratch[:, 0:128],
                                 scratch[:, 0:224], start=True, stop=True)

            # Weights split by out-channel tile; cot0 lands as two DMAs
            # (taps 0-4 / 5-9) spread over both queues so the first matmul
            # group only waits for taps 0-4 plus the first input chunk.
            w0 = wpool.tile([CI, KH * KW, 128], bf16, tag="w0")
            w1 = wpool.tile([CI, KH * KW, 128], bf16, tag="w1")
            bt = bpool.tile([128, COT], f32)

            def wslice(pos, cot):
                return w0[:, pos, :] if cot == 0 else w1[:, pos, :]

            # Input chunks per image: (padded_row0, n_blocks). The first is
            # a single block so the very first matmul group's data arrives
            # fast; block b lives in chunk CHMAP[b] at local row CHLOC[b].
            CHUNKS = [(0, 1), (RB, 2), (3 * RB, 2), (5 * RB, 2)]
            CHMAP, CHLOC = {}, {}
            b = 0
            for ci_, (r0_, nb_) in enumerate(CHUNKS):
                for j in range(nb_):
                    CHMAP[b], CHLOC[b] = ci_, j * RB
                    b += 1
            xt = {}

            def x_chunk_dma(img, ch, eng):
                r0, nb = CHUNKS[ch]
                rows = min(nb * RB + 2, HP - r0)
                t = xpool.tile([CI, rows, WP], bf16, tag=f"x{img}_{ch}")
                eng.dma_start(out=t, in_=x_d[:, img, r0:r0 + rows, :])
                xt[img, ch] = t

            # Critical path: img0/cot0 runs chunk-aligned groups [0],[1,2],
            # [3,4],[5,6]. The sync queue carries that chain in exactly the
            # order it is consumed (chunk0, taps5-8, chunk1, chunk2, cot1
            # weights); scalar serves taps0-4 + bias + chunk3 in parallel.
            x_chunk_dma(0, 0, nc.sync)
            nc.scalar.dma_start(out=w0[:, 0:5, :], in_=w_d[:, 0, 0:5, :])
            nc.sync.dma_start(out=w0[:, 5:9, :], in_=w_d[:, 0, 5:9, :])
            nc.scalar.dma_start(out=bt, in_=b_d[:, :])
            # chunk1 split across both queues: rows 0-9 serve blk1, rows
            # 10-17 complete blk2, halving its arrival time.
            r0c1, nbc1 = CHUNKS[1]
            rows_c1 = nbc1 * RB + 2
            t_c1 = xpool.tile([CI, rows_c1, WP], bf16, tag="x0_1")
            nc.sync.dma_start(out=t_c1[:, 0:10, :],
                              in_=x_d[:, 0, r0c1:r0c1 + 10, :])
            nc.scalar.dma_start(out=t_c1[:, 10:rows_c1, :],
                               in_=x_d[:, 0, r0c1 + 10:r0c1 + rows_c1, :])
            xt[0, 1] = t_c1
            x_chunk_dma(0, 2, nc.sync)
            x_chunk_dma(0, 3, nc.scalar)
            nc.sync.dma_start(out=w1, in_=w_d[:, 1, :, :])
            for img in range(1, NP_CORE):
                for ch in range(len(CHUNKS)):
                    x_chunk_dma(img, ch,
                                nc.scalar if (img + ch) % 2 else nc.sync)

            # Tap-outer over groups of row-blocks: one weight load per tap
            # per group feeds len(grp) back-to-back matmuls. 4+3 banks per
            # (img, cot) pass; the final pass ends in a single-block group
            # so the drain after the last matmul is short.
            store_q = [nc.sync, nc.scalar]
            nstore = 0

            for img in range(NP_CORE):
                for cot in range(COT):
                    first_pass = (img == 0 and cot == 0)
                    last_pass = (img == NP_CORE - 1 and cot == COT - 1)
                    if first_pass:
                        # Chunk-aligned so each group only waits for the
                        # next DMA chunk, never stalling the PE at startup.
                        groups = [[0], [1, 2], [3, 4], [5, 6]]
                    elif last_pass:
                        groups = [[0, 1, 2, 3], [4, 5], [6]]
                    else:
                        groups = [[0, 1, 2, 3], [4, 5, 6]]
                    for grp in groups:
                        pss = [ppool.tile([128, RB, W], f32, tag="ps",
                                          name=f"ps{j}")
                               for j in range(len(grp))]
                        for pos in range(KH * KW):
                            kh, kw = divmod(pos, KW)
                            wsl = wslice(pos, cot)
                            for j, blk in enumerate(grp):
                                ch, r0 = CHMAP[blk], CHLOC[blk]
                                rhs = xt[img, ch][:, r0 + kh:r0 + kh + RB,
                                                  kw:kw + W]
                                nc.tensor.matmul(
                                    pss[j], wsl, rhs,
                                    start=(pos == 0),
                                    stop=(pos == KH * KW - 1),
                                )
                        last_grp = last_pass and grp[0] == 6
                        for j, blk in enumerate(grp):
                            if last_grp:
                                # Tail: ship the final block as two
                                # half-copies, bias-added on two different
                                # engines concurrently and stored on both
                                # queues (store DMAs have ~0.6us fixed
                                # cost, so split no finer).
                                h = RB // 2
                                for q in range(2):
                                    otq = opool.tile([128, h, W], f32,
                                                     tag=f"otq{q}",
                                                     name=f"otq{q}")
                                    half = pss[j][:, q * h:(q + 1) * h, :]
                                    if q == 0:
                                        # ACT engine first in program order
                                        # so its scalar-queue guard clears
                                        # early; drains in parallel with
                                        # the DVE half below.
                                        nc.scalar.add(otq, half,
                                                      bt[:, cot:cot + 1])
                                    else:
                                        nc.vector.tensor_scalar_add(
                                            otq, half, bt[:, cot:cot + 1])
                                    r = blk * RB + q * h
                                    store_q[q % 2].dma_start(
                                        out=y_d[cot, :, img, r:r + h, :],
                                        in_=otq)
                            else:
                                ot = opool.tile([128, RB, W], f32)
                                nc.vector.tensor_scalar_add(
                                    ot, pss[j], bt[:, cot:cot + 1])
                                store_q[nstore % 2].dma_start(
                                    out=y_d[cot, :, img,
                                            blk * RB:blk * RB + RB, :],
                                    in_=ot)
                                nstore += 1

    nc.compile()
    return nc


def prep_in_maps(input, weight, bias):
    """Host-side layout prep -> one in_map per core."""
    import ml_dtypes

    bf = ml_dtypes.bfloat16
    xp = np.pad(input, ((0, 0), (0, 0), (1, 1), (1, 1))).astype(bf)
    # weight [co, ci, kh, kw] -> [ci, cot, (kh kw), cop]
    wr = np.ascontiguousarray(
        weight.transpose(1, 2, 3, 0).reshape(CI, KH * KW, COT, 128)
        .transpose(0, 2, 1, 3)).astype(bf)
    b2 = np.ascontiguousarray(bias.reshape(COT, 128).T)

    in_maps = []
    for c in range(N_CORES):
        xc = np.ascontiguousarray(
            xp[c * NP_CORE:(c + 1) * NP_CORE].transpose(1, 0, 2, 3))
        in_maps.append({"x": xc, "w": wr, "b2": b2})
    return in_maps


def kernel(input, weight, bias):
    input = np.asarray(input, dtype=np.float32)
    weight = np.asarray(weight, dtype=np.float32)
    bias = np.asarray(bias, dtype=np.float32)

    if "nc" not in _CACHE:
        _CACHE["nc"] = _build_program()
    nc = _CACHE["nc"]

    from concourse.bass_utils import run_bass_kernel_spmd

    in_maps = prep_in_maps(input, weight, bias)
    res = run_bass_kernel_spmd(nc, in_maps, core_ids=list(range(N_CORES)))

    out = np.empty((N, CO, H, W), dtype=np.float32)
    for c in range(N_CORES):
        y = res.results[c]["y"]  # [COT, 128, NP_CORE, H, W]
        out[c * NP_CORE:(c + 1) * NP_CORE] = (
            y.transpose(2, 0, 1, 3, 4).reshape(NP_CORE, CO, H, W))
    return out
